# revision 14
# baseline (speedup 1.0000x reference)
"""Trainium2 Bass kernel for nn_DecoderLSTMAttention.

Math (exploiting that the reference softmax is over a singleton axis, so
attention weights are identically 1 and context == features broadcast):

    x        = concat([features[:, None, :], embed[captions[:, :-1]]], 1)   # (B,S,E)
    xg       = x @ W_ih.T + (b_ih + b_hh)                                   # (B,S,4H)
    h_t, c_t = lstm_step(xg_t, h_{t-1}, c_{t-1}; W_hh)                      # gates i,f,g,o
    out      = (lstm_out + features[:, None, :]) @ out_W.T + out_b          # (B,S,V)

Sharding: pure data-parallel over batch B=64 across 8 cores (8 batches per
core, no collectives).  Embedding gather + weight transposes/casts happen on
the host; everything else on device.

Device layout (per core, Bc=8, H=1024, G=4H=4096, T=S*Bc tokens, token
index tau = 128*(t//16) + 8*(t%16) + b):

All matmuls keep the SMALL operand stationary (LDWEIGHTS cost scales with
stationary columns) and stream the BIG weight matrix as the moving operand:

  - GEMM A:  stationary xT token-tiles [128e x 128tok], moving W_ih^T
    panels [128e x 512g] streamed from DRAM; PSUM [128tok x 512g]; a
    K=1 ones-row matmul injects the (b_ih+b_hh) bias.  Result xg kept
    on-chip as [128tok, 4tt, 4096g] bf16.
  - recurrence: stationary hT k-chunks [128h x 8b] (tiny 6.7ns loads),
    moving W_hh^T [128h x 512g] resident in SBUF.  Gate columns are
    host-permuted into per-h-block groups [i_j f_j o_j g_j] (128 each)
    so PSUM bank j yields gates for h dims [128j,128j+128): elementwise
    + PE-transpose of each h block pipelines behind the next bank's
    matmuls, and step t+1's k=j stationary is ready as soon as block j
    finishes.
  - GEMM B: stationary combT token-tiles (lstm_out+features, bf16),
    moving out_W^T panels streamed from DRAM, fp32 PSUM -> SBUF -> DRAM
    out [T, V].
"""

import numpy as np
import ml_dtypes

import concourse.bass as bass
import concourse.tile as tile
from concourse import bacc, mybir
from concourse.bass_utils import run_bass_kernel_spmd

BF16 = mybir.dt.bfloat16
F32 = mybir.dt.float32
AF = mybir.ActivationFunctionType

P = 128
BC = 8          # batches per core
H = 1024        # hidden = embed
G = 4 * H       # gates
NK = H // P     # 8 h-chunks
NG = G // 512   # 8 gate column chunks of 512
NTT = 4         # token tiles of 128 (= S*BC/128)

# gate-column permutation: chunk j (512 cols) = [i_j, f_j, o_j, g_j] where
# x_j covers h dims [128j, 128j+128).  Original W rows are [i, f, g, o].
PERM = np.concatenate([
    np.concatenate([gate * H + np.arange(P) + j * P for gate in (0, 1, 3, 2)])
    for j in range(NK)])


def emit_body(tc, io, S, V):
    """Emit the per-core program. io maps logical names -> DRAM APs."""
    nc = tc.nc
    T = S * BC

    xt_d, wih_d, whh_d, outw_d = io["xt"], io["wih"], io["whh"], io["outw"]
    feat_d, bias_d, out_d = io["feat"], io["biasg"], io["out"]

    import contextlib
    ctx = contextlib.ExitStack()
    with ctx:
        state = ctx.enter_context(tc.tile_pool(name="state", bufs=1))
        wih_pool = ctx.enter_context(tc.tile_pool(name="wih", bufs=8))
        outw_pool = ctx.enter_context(tc.tile_pool(name="outw", bufs=24))
        gsb_pool = ctx.enter_context(tc.tile_pool(name="gsb", bufs=4))
        xgs_pool = ctx.enter_context(tc.tile_pool(name="xgs", bufs=12))
        tmp_pool = ctx.enter_context(tc.tile_pool(name="tmps", bufs=6))
        stage_pool = ctx.enter_context(tc.tile_pool(name="stage", bufs=6))
        mm_pool = ctx.enter_context(tc.tile_pool(name="mm", bufs=4, space="PSUM"))
        gps_pool = ctx.enter_context(tc.tile_pool(name="gps", bufs=3, space="PSUM"))
        tr_pool = ctx.enter_context(tc.tile_pool(name="tr", bufs=1, space="PSUM"))

        # ---- resident tensors ----
        whh_sb = state.tile([P, NK, G], BF16, tag="whh_sb")
        nc.sync.dma_start(whh_sb[:], whh_d.rearrange("(k p) g -> p k g", p=P))
        xt_sb = state.tile([P, NK, T], BF16, tag="xt_sb")
        nc.sync.dma_start(xt_sb[:], xt_d.rearrange("(k p) t -> p k t", p=P))
        feat_sb = state.tile([P, NK, BC], F32, tag="feat_sb")
        nc.sync.dma_start(feat_sb[:], feat_d[:])
        bias_sb = state.tile([1, G], BF16, tag="bias_sb")
        nc.sync.dma_start(bias_sb[:], bias_d[:])

        xg_sb = state.tile([P, NTT, G], BF16, tag="xg_sb")
        lstm_sb = state.tile([P, NK, S, BC], BF16, tag="lstm_sb")
        c_sb = state.tile([BC, H], F32, tag="c_sb")
        nc.any.memset(c_sb[:], 0.0)
        h_sb = state.tile([BC, H], BF16, tag="h_sb")
        ones_sb = state.tile([1, P], BF16, tag="ones_sb")
        nc.any.memset(ones_sb[:], 1.0)
        ident8 = state.tile([BC, BC], BF16, tag="ident8")
        nc.sync.dma_start(ident8[:], io["ident8"][:])

        # ---- GEMM A: xg[tok, g] = x[tok, :] @ W_ih^T + bias ----
        # g outer so W_ih^T panels stream once; tok inner reloads the small
        # xT stationaries (4 per panel), all hidden under N=512 streams.
        for g in range(NG):
            pans = [wih_pool.tile([P, 512], BF16, tag="wih_p", name="wih_p")
                    for _ in range(NK)]
            for kk in range(NK):
                nc.sync.dma_start(
                    pans[kk][:], wih_d[kk * P:(kk + 1) * P, g * 512:(g + 1) * 512])
            for tt in range(NTT):
                ps = mm_pool.tile([P, 512], F32, tag="ps")
                nc.tensor.matmul(ps[:], ones_sb[:],
                                 bias_sb[:, g * 512:(g + 1) * 512],
                                 start=True, stop=False)
                for kk in range(NK):
                    nc.tensor.matmul(
                        ps[:], xt_sb[:, kk, tt * P:(tt + 1) * P],
                        pans[kk][:], start=False, stop=(kk == NK - 1))
                nc.any.tensor_copy(xg_sb[:, tt, g * 512:(g + 1) * 512], ps[:])

        # ---- recurrence ----
        # step t consumes xg rows [8u:8u+8] of token-tile tt (t = 16*tt + u)
        # and hT k-chunks lstm_sb[:, k, t-1, :]; PSUM bank j yields gate
        # block j = [i f o g] for h dims [128j, 128j+128).
        def emit_transpose(t, j):
            # transpose h block j -> hT chunk j for step t+1 / GEMM B
            tp = tr_pool.tile([P, BC], BF16, tag="tp")
            nc.tensor.transpose(tp[:], h_sb[:, j * P:(j + 1) * P], ident8[:])
            nc.scalar.activation(lstm_sb[:, j, t, :], tp[:], AF.Identity)

        # xg slices live at partition offset 8*(t%16); engines need 32-aligned
        # partition bases, so a (DMA-engine) SB->SB gather stages each block
        # at partition 0, prefetched LEAD block-slots ahead.
        LEAD = 8
        xgs = {}

        def emit_xg_fetch(t, j):
            if t >= S:
                return
            tt, u = divmod(t, 16)
            xt_tile = xgs_pool.tile([BC, 512], BF16, tag="xgs", name="xgs")
            nc.sync.dma_start(
                xt_tile[:], xg_sb[u * BC:(u + 1) * BC, tt, j * 512:(j + 1) * 512])
            xgs[(t, j)] = xt_tile

        for j in range(LEAD):
            emit_xg_fetch(0, j)

        for t in range(S):
            tt, u = divmod(t, 16)
            for j in range(NG):
                gsl = slice(j * 512, (j + 1) * 512)
                jn = j + LEAD
                emit_xg_fetch(t + jn // NG, jn % NG)
                xg_t = xgs.pop((t, j))
                gt = gsb_pool.tile([BC, 512], F32, tag="gt")
                if t == 0:
                    nc.any.tensor_copy(gt[:], xg_t[:])
                else:
                    gps = gps_pool.tile([BC, 512], F32, tag="gps")
                    for kk in range(NK):
                        nc.tensor.matmul(
                            gps[:], lstm_sb[:, kk, t - 1, :],
                            whh_sb[:, kk, gsl],
                            start=(kk == 0), stop=(kk == NK - 1))
                    nc.vector.tensor_add(gt[:], gps[:], xg_t[:])
                # transposes lag 2 blocks so the (in-order) PE never waits on
                # the current block's elementwise chain
                if j >= 2:
                    emit_transpose(t, j - 2)
                sg = tmp_pool.tile([BC, 384], F32, tag="sg")
                nc.scalar.activation(sg[:], gt[:, 0:384], AF.Sigmoid)
                tg = tmp_pool.tile([BC, P], F32, tag="tg")
                nc.scalar.activation(tg[:], gt[:, 384:512], AF.Tanh)
                csl = c_sb[:, j * P:(j + 1) * P]
                ig = tmp_pool.tile([BC, P], F32, tag="ig")
                nc.vector.tensor_mul(ig[:], sg[:, 0:P], tg[:])
                nc.vector.tensor_mul(csl, sg[:, P:2 * P], csl)
                nc.vector.tensor_add(csl, csl, ig[:])
                tc_t = tmp_pool.tile([BC, P], F32, tag="tc_t")
                nc.scalar.activation(tc_t[:], csl, AF.Tanh)
                hbl = h_sb[:, j * P:(j + 1) * P]
                nc.vector.tensor_mul(hbl, sg[:, 2 * P:3 * P], tc_t[:])
            emit_transpose(t, NG - 2)
            emit_transpose(t, NG - 1)

        # ---- combined = lstm_out + features (broadcast over t), in place ----
        for kk in range(NK):
            nc.vector.tensor_add(
                lstm_sb[:, kk], lstm_sb[:, kk],
                feat_sb[:, kk, None, :].to_broadcast([P, S, BC]))

        # ---- GEMM B: out[tau, v] = sum_h combT[h, tau] * out_W^T[h, v] ----
        vtiles = []
        off = 0
        while off < V:
            sz = min(512, V - off)
            vtiles.append((off, sz))
            off += sz
        for off, sz in vtiles:
            pans = [outw_pool.tile([P, 512], BF16, tag="outw_p", name="outw_p")
                    for _ in range(NK)]
            for kk in range(NK):
                nc.sync.dma_start(pans[kk][:, :sz],
                                  outw_d[kk * P:(kk + 1) * P, off:off + sz])
            for m in range(NTT):
                ps = mm_pool.tile([P, 512], F32, tag="ps")
                for kk in range(NK):
                    nc.tensor.matmul(
                        ps[:, :sz],
                        lstm_sb[:, kk, 16 * m:16 * (m + 1), :],
                        pans[kk][:, :sz],
                        start=(kk == 0), stop=(kk == NK - 1))
                st = stage_pool.tile([P, 512], F32, tag="st")
                nc.any.tensor_copy(st[:, :sz], ps[:, :sz])
                nc.sync.dma_start(
                    out_d[m * P:(m + 1) * P, off:off + sz], st[:, :sz])


# ------------------------------------------------------------------ host ----


def host_prep(features, captions, embed_table, W_ih, W_hh, b_ih, b_hh,
              out_W, S, V):
    """Shared weights + per-core input shards."""
    bf = ml_dtypes.bfloat16
    b = (np.asarray(b_ih, np.float32) + np.asarray(b_hh, np.float32))[PERM]
    biasg = np.ascontiguousarray(b.reshape(1, G)).astype(bf)        # [1, 4096]
    wihT = np.asarray(W_ih, np.float32).T[:, PERM].astype(bf)
    whhT = np.asarray(W_hh, np.float32).T[:, PERM].astype(bf)
    outwT = np.asarray(out_W, np.float32).T.astype(bf)

    features = np.asarray(features, np.float32)
    cap = np.asarray(captions).astype(np.int64)
    x = np.concatenate(
        [features[:, None, :], np.asarray(embed_table, np.float32)[cap[:, :S - 1]]],
        axis=1)                                                     # (B, S, E)

    shards = []
    B = features.shape[0]
    for c in range(B // BC):
        xc = x[c * BC:(c + 1) * BC]                                 # (8, S, E)
        xT = xc.transpose(2, 1, 0).reshape(H, S * BC).astype(bf)
        fc = features[c * BC:(c + 1) * BC]
        featT = np.ascontiguousarray(fc.T.reshape(NK, P, BC).transpose(1, 0, 2))
        shards.append({"xt": xT, "wih": wihT, "whh": whhT, "outw": outwT,
                       "feat": featT, "biasg": biasg,
                       "ident8": np.eye(BC, dtype=bf)})
    return shards


def build_program(S, V):
    nc = bacc.Bacc("TRN2", target_bir_lowering=False, debug=False,
                   enable_asserts=False)
    T = S * BC
    io = {
        "xt": nc.dram_tensor("xt", [H, T], BF16, kind="ExternalInput").ap(),
        "wih": nc.dram_tensor("wih", [H, G], BF16, kind="ExternalInput").ap(),
        "whh": nc.dram_tensor("whh", [H, G], BF16, kind="ExternalInput").ap(),
        "outw": nc.dram_tensor("outw", [H, V], BF16, kind="ExternalInput").ap(),
        "feat": nc.dram_tensor("feat", [P, NK, BC], F32, kind="ExternalInput").ap(),
        "biasg": nc.dram_tensor("biasg", [1, G], BF16, kind="ExternalInput").ap(),
        "ident8": nc.dram_tensor("ident8", [BC, BC], BF16, kind="ExternalInput").ap(),
        "out": nc.dram_tensor("out", [T, V], F32, kind="ExternalOutput").ap(),
    }
    with tile.TileContext(nc) as tc:
        emit_body(tc, io, S, V)
    nc.compile()
    return nc


_CACHE = {}


def _get_program(S, V):
    key = (S, V)
    if key not in _CACHE:
        _CACHE[key] = build_program(S, V)
    return _CACHE[key]


def kernel(features, captions, embed_table, W_ih, W_hh, b_ih, b_hh,
           attn_W, attn_b, score_W, score_b, out_W, out_b):
    S = np.asarray(captions).shape[1]
    V = np.asarray(out_W).shape[0]
    B = np.asarray(features).shape[0]
    shards = host_prep(features, captions, embed_table, W_ih, W_hh,
                       b_ih, b_hh, out_W, S, V)
    nc = _get_program(S, V)
    res = run_bass_kernel_spmd(nc, shards, core_ids=list(range(len(shards))))
    out = np.empty((B, S, V), np.float32)
    for c in range(len(shards)):
        oc = res.results[c]["out"].reshape(S, BC, V).transpose(1, 0, 2)
        out[c * BC:(c + 1) * BC] = oc
    out_b = np.asarray(out_b, np.float32)
    if np.any(out_b):
        out += out_b
    return out


# revision 17
# speedup vs baseline: 1.5610x; 1.5610x over previous
"""Trainium2 Bass kernel for nn_DecoderLSTMAttention.

Math (exploiting that the reference softmax is over a singleton axis, so
attention weights are identically 1 and context == features broadcast):

    x        = concat([features[:, None, :], embed[captions[:, :-1]]], 1)   # (B,S,E)
    xg       = x @ W_ih.T + (b_ih + b_hh)                                   # (B,S,4H)
    h_t, c_t = lstm_step(xg_t, h_{t-1}, c_{t-1}; W_hh)                      # gates i,f,g,o
    out      = (lstm_out + features[:, None, :]) @ out_W.T + out_b          # (B,S,V)

Sharding: pure data-parallel over batch B=64 across 8 cores (8 batches per
core, no collectives).  Embedding gather + weight transposes/casts happen on
the host; everything else on device.

Device layout (per core, Bc=8, H=1024, G=4H=4096, T=S*Bc tokens, token
index tau = 128*(t//16) + 8*(t%16) + b):

All matmuls keep the SMALL operand stationary (LDWEIGHTS cost scales with
stationary columns) and stream the BIG weight matrix as the moving operand:

  - GEMM A:  stationary xT token-tiles [128e x 128tok], moving W_ih^T
    panels [128e x 512g] streamed from DRAM; PSUM [128tok x 512g]; a
    K=1 ones-row matmul injects the (b_ih+b_hh) bias.  Result xg kept
    on-chip as [128tok, 4tt, 4096g] bf16.
  - recurrence: stationary hT k-chunks [128h x 8b] (tiny 6.7ns loads),
    moving W_hh^T [128h x 512g] resident in SBUF.  Gate columns are
    host-permuted into per-h-block groups [i_j f_j o_j g_j] (128 each)
    so PSUM bank j yields gates for h dims [128j,128j+128): elementwise
    + PE-transpose of each h block pipelines behind the next bank's
    matmuls, and step t+1's k=j stationary is ready as soon as block j
    finishes.
  - GEMM B: stationary combT token-tiles (lstm_out+features, bf16),
    moving out_W^T panels streamed from DRAM, fp32 PSUM -> SBUF -> DRAM
    out [T, V].
"""

import numpy as np
import ml_dtypes

import concourse.bass as bass
import concourse.tile as tile
from concourse import bacc, mybir
from concourse.bass_utils import run_bass_kernel_spmd

BF16 = mybir.dt.bfloat16
F32 = mybir.dt.float32
AF = mybir.ActivationFunctionType

P = 128
BC = 8          # batches per core
H = 1024        # hidden = embed
G = 4 * H       # gates
NK = H // P     # 8 h-chunks
NG = G // 512   # 8 gate column chunks of 512
NTT = 4         # token tiles of 128 (= S*BC/128)

# gate-column permutation: chunk j (512 cols) = [i_j, f_j, o_j, g_j] where
# x_j covers h dims [128j, 128j+128).  Original W rows are [i, f, g, o].
PERM = np.concatenate([
    np.concatenate([gate * H + np.arange(P) + j * P for gate in (0, 1, 3, 2)])
    for j in range(NK)])


def emit_body(tc, io, S, V):
    """Emit the per-core program. io maps logical names -> DRAM APs."""
    nc = tc.nc
    T = S * BC

    xt_d, wih_d, whh_d, outw_d = io["xt"], io["wih"], io["whh"], io["outw"]
    feat_d, bias_d, out_d = io["feat"], io["biasg"], io["out"]

    import contextlib
    ctx = contextlib.ExitStack()
    with ctx:
        state = ctx.enter_context(tc.tile_pool(name="state", bufs=1))
        wih_pool = ctx.enter_context(tc.tile_pool(name="wih", bufs=8))
        outw_pool = ctx.enter_context(tc.tile_pool(name="outw", bufs=24))
        gsb_pool = ctx.enter_context(tc.tile_pool(name="gsb", bufs=4))
        xgs_pool = ctx.enter_context(tc.tile_pool(name="xgs", bufs=12))
        tmp_pool = ctx.enter_context(tc.tile_pool(name="tmps", bufs=6))
        stage_pool = ctx.enter_context(tc.tile_pool(name="stage", bufs=5))
        mm_pool = ctx.enter_context(tc.tile_pool(name="mm", bufs=4, space="PSUM"))
        gps_pool = ctx.enter_context(tc.tile_pool(name="gps", bufs=3, space="PSUM"))
        tr_pool = ctx.enter_context(tc.tile_pool(name="tr", bufs=1, space="PSUM"))

        # ---- resident tensors ----
        whh_sb = state.tile([P, NK, G], BF16, tag="whh_sb")
        nc.sync.dma_start(whh_sb[:], whh_d.rearrange("(k p) g -> p k g", p=P))
        xt_sb = state.tile([P, NK, T], BF16, tag="xt_sb")
        nc.sync.dma_start(xt_sb[:], xt_d.rearrange("(k p) t -> p k t", p=P))
        feat_sb = state.tile([P, NK, BC], F32, tag="feat_sb")
        nc.sync.dma_start(feat_sb[:], feat_d[:])
        bias_sb = state.tile([1, G], BF16, tag="bias_sb")
        nc.sync.dma_start(bias_sb[:], bias_d[:])

        xg_sb = state.tile([P, NTT, G], BF16, tag="xg_sb")
        lstm_sb = state.tile([P, NK, S, BC], BF16, tag="lstm_sb")
        c_sb = state.tile([BC, H], F32, tag="c_sb")
        nc.any.memset(c_sb[:], 0.0)
        h_sb = state.tile([BC, H], BF16, tag="h_sb")
        ones_sb = state.tile([1, P], BF16, tag="ones_sb")
        nc.any.memset(ones_sb[:], 1.0)
        ident8 = state.tile([BC, BC], BF16, tag="ident8")
        nc.sync.dma_start(ident8[:], io["ident8"][:])

        # ---- GEMM A: xg[tok, g] = x[tok, :] @ W_ih^T + bias ----
        # g outer so W_ih^T panels stream once; tok inner reloads the small
        # xT stationaries (4 per panel), all hidden under N=512 streams.
        for g in range(NG):
            pans = [wih_pool.tile([P, 512], BF16, tag="wih_p", name="wih_p")
                    for _ in range(NK)]
            for kk in range(NK):
                nc.sync.dma_start(
                    pans[kk][:], wih_d[kk * P:(kk + 1) * P, g * 512:(g + 1) * 512])
            for tt in range(NTT):
                ps = mm_pool.tile([P, 512], F32, tag="ps")
                nc.tensor.matmul(ps[:], ones_sb[:],
                                 bias_sb[:, g * 512:(g + 1) * 512],
                                 start=True, stop=False)
                for kk in range(NK):
                    nc.tensor.matmul(
                        ps[:], xt_sb[:, kk, tt * P:(tt + 1) * P],
                        pans[kk][:], start=False, stop=(kk == NK - 1))
                nc.any.tensor_copy(xg_sb[:, tt, g * 512:(g + 1) * 512], ps[:])

        # ---- recurrence ----
        # step t consumes xg rows [8u:8u+8] of token-tile tt (t = 16*tt + u)
        # and hT k-chunks lstm_sb[:, k, t-1, :]; PSUM bank j yields gate
        # block j = [i f o g] for h dims [128j, 128j+128).
        def emit_transpose(t, j):
            # transpose h block j -> hT chunk j for step t+1 / GEMM B
            tp = tr_pool.tile([P, BC], BF16, tag="tp")
            nc.tensor.transpose(tp[:], h_sb[:, j * P:(j + 1) * P], ident8[:])
            nc.scalar.activation(lstm_sb[:, j, t, :], tp[:], AF.Identity)

        # xg slices live at partition offset 8*(t%16); engines need 32-aligned
        # partition bases, so a (DMA-engine) SB->SB gather stages each block
        # at partition 0, prefetched LEAD block-slots ahead.
        LEAD = 8
        xgs = {}

        def emit_xg_fetch(t, j):
            if t >= S:
                return
            tt, u = divmod(t, 16)
            xt_tile = xgs_pool.tile([BC, 512], BF16, tag="xgs", name="xgs")
            nc.sync.dma_start(
                xt_tile[:], xg_sb[u * BC:(u + 1) * BC, tt, j * 512:(j + 1) * 512])
            xgs[(t, j)] = xt_tile

        for j in range(LEAD):
            emit_xg_fetch(0, j)

        for t in range(S):
            tt, u = divmod(t, 16)
            for j in range(NG):
                gsl = slice(j * 512, (j + 1) * 512)
                jn = j + LEAD
                emit_xg_fetch(t + jn // NG, jn % NG)
                xg_t = xgs.pop((t, j))
                gt = gsb_pool.tile([BC, 512], F32, tag="gt")
                if t == 0:
                    nc.any.tensor_copy(gt[:], xg_t[:])
                else:
                    gps = gps_pool.tile([BC, 512], F32, tag="gps")
                    for kk in range(NK):
                        nc.tensor.matmul(
                            gps[:], lstm_sb[:, kk, t - 1, :],
                            whh_sb[:, kk, gsl],
                            start=(kk == 0), stop=(kk == NK - 1))
                    nc.vector.tensor_add(gt[:], gps[:], xg_t[:])
                # transposes lag 2 blocks so the (in-order) PE never waits on
                # the current block's elementwise chain
                if j >= 2:
                    emit_transpose(t, j - 2)
                sg = tmp_pool.tile([BC, 384], F32, tag="sg")
                nc.scalar.activation(sg[:], gt[:, 0:384], AF.Sigmoid)
                tg = tmp_pool.tile([BC, P], F32, tag="tg")
                nc.scalar.activation(tg[:], gt[:, 384:512], AF.Tanh)
                csl = c_sb[:, j * P:(j + 1) * P]
                ig = tmp_pool.tile([BC, P], F32, tag="ig")
                nc.vector.tensor_mul(ig[:], sg[:, 0:P], tg[:])
                nc.vector.tensor_mul(csl, sg[:, P:2 * P], csl)
                nc.vector.tensor_add(csl, csl, ig[:])
                tc_t = tmp_pool.tile([BC, P], F32, tag="tc_t")
                nc.scalar.activation(tc_t[:], csl, AF.Tanh)
                hbl = h_sb[:, j * P:(j + 1) * P]
                nc.vector.tensor_mul(hbl, sg[:, 2 * P:3 * P], tc_t[:])
            emit_transpose(t, NG - 2)
            emit_transpose(t, NG - 1)

        # ---- combined = lstm_out + features (broadcast over t), in place ----
        for kk in range(NK):
            nc.vector.tensor_add(
                lstm_sb[:, kk], lstm_sb[:, kk],
                feat_sb[:, kk, None, :].to_broadcast([P, S, BC]))

        # ---- GEMM B: out[tau, v] = sum_h combT[h, tau] * out_W^T[h, v] ----
        vtiles = []
        off = 0
        while off < V:
            sz = min(512, V - off)
            vtiles.append((off, sz))
            off += sz
        for off, sz in vtiles:
            pans = [outw_pool.tile([P, 512], BF16, tag="outw_p", name="outw_p")
                    for _ in range(NK)]
            for kk in range(NK):
                nc.sync.dma_start(pans[kk][:, :sz],
                                  outw_d[kk * P:(kk + 1) * P, off:off + sz])
            for m in range(NTT):
                ps = mm_pool.tile([P, 512], F32, tag="ps")
                for kk in range(NK):
                    nc.tensor.matmul(
                        ps[:, :sz],
                        lstm_sb[:, kk, 16 * m:16 * (m + 1), :],
                        pans[kk][:, :sz],
                        start=(kk == 0), stop=(kk == NK - 1))
                st = stage_pool.tile([P, 512], F32, tag="st")
                nc.any.tensor_copy(st[:, :sz], ps[:, :sz])
                nc.sync.dma_start(
                    out_d[m * P:(m + 1) * P, off:off + sz], st[:, :sz])


# ------------------------------------------------------------------ host ----


def host_prep(features, captions, embed_table, W_ih, W_hh, b_ih, b_hh,
              out_W, S, V):
    """Shared weights + per-core input shards."""
    bf = ml_dtypes.bfloat16
    b = (np.asarray(b_ih, np.float32) + np.asarray(b_hh, np.float32))[PERM]
    biasg = np.ascontiguousarray(b.reshape(1, G)).astype(bf)        # [1, 4096]
    wihT = np.asarray(W_ih, np.float32).T[:, PERM].astype(bf)
    whhT = np.asarray(W_hh, np.float32).T[:, PERM].astype(bf)
    outwT = np.asarray(out_W, np.float32).T.astype(bf)

    features = np.asarray(features, np.float32)
    cap = np.asarray(captions).astype(np.int64)
    x = np.concatenate(
        [features[:, None, :], np.asarray(embed_table, np.float32)[cap[:, :S - 1]]],
        axis=1)                                                     # (B, S, E)

    shards = []
    B = features.shape[0]
    for c in range(B // BC):
        xc = x[c * BC:(c + 1) * BC]                                 # (8, S, E)
        xT = xc.transpose(2, 1, 0).reshape(H, S * BC).astype(bf)
        fc = features[c * BC:(c + 1) * BC]
        featT = np.ascontiguousarray(fc.T.reshape(NK, P, BC).transpose(1, 0, 2))
        shards.append({"xt": xT, "wih": wihT, "whh": whhT, "outw": outwT,
                       "feat": featT, "biasg": biasg,
                       "ident8": np.eye(BC, dtype=bf)})
    return shards


def build_program(S, V, k_bodies=1):
    """k_bodies > 1 emits the kernel body k times back-to-back in one NEFF
    (same inputs, same output; idempotent) so steady-state per-execution
    time can be measured with launch/dispatch overhead amortized."""
    nc = bacc.Bacc("TRN2", target_bir_lowering=False, debug=False,
                   enable_asserts=False)
    T = S * BC
    io = {
        "xt": nc.dram_tensor("xt", [H, T], BF16, kind="ExternalInput").ap(),
        "wih": nc.dram_tensor("wih", [H, G], BF16, kind="ExternalInput").ap(),
        "whh": nc.dram_tensor("whh", [H, G], BF16, kind="ExternalInput").ap(),
        "outw": nc.dram_tensor("outw", [H, V], BF16, kind="ExternalInput").ap(),
        "feat": nc.dram_tensor("feat", [P, NK, BC], F32, kind="ExternalInput").ap(),
        "biasg": nc.dram_tensor("biasg", [1, G], BF16, kind="ExternalInput").ap(),
        "ident8": nc.dram_tensor("ident8", [BC, BC], BF16, kind="ExternalInput").ap(),
        "out": nc.dram_tensor("out", [T, V], F32, kind="ExternalOutput").ap(),
    }
    with tile.TileContext(nc) as tc:
        for _ in range(k_bodies):
            emit_body(tc, io, S, V)
    nc.compile()
    return nc


_CACHE = {}


def _get_program(S, V):
    key = (S, V)
    if key not in _CACHE:
        _CACHE[key] = build_program(S, V)
    return _CACHE[key]


def kernel(features, captions, embed_table, W_ih, W_hh, b_ih, b_hh,
           attn_W, attn_b, score_W, score_b, out_W, out_b):
    S = np.asarray(captions).shape[1]
    V = np.asarray(out_W).shape[0]
    B = np.asarray(features).shape[0]
    shards = host_prep(features, captions, embed_table, W_ih, W_hh,
                       b_ih, b_hh, out_W, S, V)
    nc = _get_program(S, V)
    res = run_bass_kernel_spmd(nc, shards, core_ids=list(range(len(shards))))
    out = np.empty((B, S, V), np.float32)
    for c in range(len(shards)):
        oc = res.results[c]["out"].reshape(S, BC, V).transpose(1, 0, 2)
        out[c * BC:(c + 1) * BC] = oc
    out_b = np.asarray(out_b, np.float32)
    if np.any(out_b):
        out += out_b
    return out
